# revision 17
# baseline (speedup 1.0000x reference)
"""Trainium2 Bass kernel for nn_Attention_update (additive attention pooling).

reference math (per example b):
    pre[s,d] = enc[b] @ W1e^T + (W1h @ h[b] + b1)      # [S, D]
    e[s]     = tanh(pre) @ W2[0]                        # [S]
    alpha    = softmax(e);  ctx = alpha @ enc[b]        # [DK]

Sharding: data-parallel over batch B=64 across 8 cores (8 examples/core),
same SPMD program on every core, no collectives.

Per-core kernel, fp8 DoubleRow score path in [d-part, s-free] layout:
  - scores: stationary = W1e chunks quantized to fp8e4 (pre-scaled x64 to
    dodge subnormal truncation), moving = enc in fp8e4 (pre-scaled x8),
    perf_mode=DoubleRow -> K=256 per instruction, ~2x bf16 MAC rate.
    pre lands as [128 d-part, 512 s-free] PSUM tiles (8 d-blocks x 4 j).
  - bias+tanh fused on ScalarE: tanh(pre/512 + hv[d]) with the per-example
    hv = W1h@h+b1 column as the ACT per-partition bias operand; output
    written bf16 straight into SBUF.
  - e[s] = sum_d W2_d tanh: K=128 matmul chain over the 8 d-blocks with
    W2 column stationary -> e as [1, 512] PSUM; a strided DMA transposes
    it into eT [128 s-part, 16]; ACT exp with accum_out gives p and its
    per-partition sum; ones-matmul + reciprocal finish the softmax scale.
  - ctx = sum_t p[:,t] (x) encN-tile chains in bf16, scaled by 1/sum.
    ctx of example b is emitted during the score phase of example b+1.
enc is supplied from host in fp8 (scores, 16.8MB) and bf16 (context,
33.5MB): ~50MB/core @ ~285GB/s hides under the PE-bound runtime.
Accuracy: fp8 quantization of enc/W1e gives rel err ~1e-2 on the fixed
harness inputs (vs 2e-2 gate); verified by numpy simulation + HW run.
"""
import os
import numpy as np
import ml_dtypes

import concourse.bass as bass
import concourse.mybir as mybir
import concourse.tile as tile
from concourse import bacc
from concourse.bass import ts
from concourse.bass_utils import run_bass_kernel_spmd

AF = mybir.ActivationFunctionType
ALU = mybir.AluOpType
F32 = mybir.dt.float32
F32R = mybir.dt.float32r
BF16 = mybir.dt.bfloat16
FP8 = mybir.dt.float8e4
DRW = mybir.MatmulPerfMode.DoubleRow

N_CORES = 8
B, S, DK, D = 64, 2048, 1024, 1024
BC = B // N_CORES          # examples per core
MC = D // 128              # m chunks (hidden dim)
NDB = D // 128             # d blocks
NJ = DK // 256             # DoubleRow k chunks (K=256 each)
NT = S // 512              # s512 tiles per example
NST = S // 128             # s tiles for ctx
SW = 64.0                  # W1e pre-quantization scale
SE = 8.0                   # enc pre-quantization scale
SCALE = 1.0 / (SW * SE)    # descale folded into ACT


CT_ERED = os.environ.get("CT_ERED", "0") == "1"
CT_CTX = os.environ.get("CT_CTX", "0") == "1"


def build_kernel(reps: int = 1):
    nc = bacc.Bacc(None)

    encdr = nc.dram_tensor("encdr", [128, 2 * NJ, BC * S], FP8, kind="ExternalInput")
    encn = nc.dram_tensor("encn", [BC * S, DK], BF16, kind="ExternalInput")
    w1edr = nc.dram_tensor("w1edr", [128, 2 * NJ, D], FP8, kind="ExternalInput")
    w1hT = nc.dram_tensor("w1hT", [128, MC, D], F32R, kind="ExternalInput")
    hT = nc.dram_tensor("hT", [128, MC, BC], F32R, kind="ExternalInput")
    b1bc = nc.dram_tensor("b1bc", [128, NDB * BC], F32, kind="ExternalInput")
    w2dp = nc.dram_tensor("w2dp", [128, NDB, 32], BF16, kind="ExternalInput")
    out_d = nc.dram_tensor("out", [BC, DK], F32, kind="ExternalOutput")

    with tile.TileContext(nc) as tc:
        with (
            tc.tile_pool(name="consts", bufs=1) as consts,
            tc.tile_pool(name="smalls", bufs=4) as smalls,
            tc.tile_pool(name="prep", bufs=3 if (CT_ERED or CT_CTX) else 4,
                         space="PSUM") as prep,
            tc.tile_pool(name="epsp", bufs=1, space="PSUM") as epsp,
            tc.tile_pool(name="miscps", bufs=1, space="PSUM") as miscps,
            tc.tile_pool(name="sumps", bufs=1, space="PSUM") as sumps,
        ):
            # ---- constants / parameters ----
            w1e_sb = consts.tile([128, 2 * NJ, D], FP8)
            nc.sync.dma_start(out=w1e_sb, in_=w1edr[:, :, :])
            w2_sb = consts.tile([128, NDB, 32], BF16)
            nc.sync.dma_start(out=w2_sb, in_=w2dp[:, :, :])
            ones_col = consts.tile([128, 1], F32)
            nc.vector.memset(ones_col, 1.0)
            ones4 = consts.tile([4, 1], F32)
            nc.vector.memset(ones4, 1.0)
            hv_sb = consts.tile([128, NDB * BC], F32)

            # ---- startup: hv[d, b] = (W1h @ h + b1) in d-part layout ----
            with tc.tile_pool(name="w1hp", bufs=1) as w1hp:
                w1hT_sb = w1hp.tile([128, MC, D], F32R)
                nc.sync.dma_start(out=w1hT_sb, in_=w1hT[:, :, :])
                hT_sb = w1hp.tile([128, MC, BC], F32R)
                nc.sync.dma_start(out=hT_sb, in_=hT[:, :, :])
                b1_sb = w1hp.tile([128, NDB * BC], F32)
                nc.sync.dma_start(out=b1_sb, in_=b1bc[:, :])
                hv_ps = miscps.tile([128, NDB * BC], F32, tag="misc")
                for db in range(NDB):
                    sl = slice(db * BC, (db + 1) * BC)
                    for mc in range(MC):
                        nc.tensor.matmul(
                            hv_ps[:, sl], w1hT_sb[:, mc, ts(db, 128)],
                            hT_sb[:, mc, :], start=(mc == 0), stop=(mc == MC - 1))
                nc.vector.tensor_add(hv_sb, hv_ps, b1_sb)

            with (
                tc.tile_pool(name="edram", bufs=2, space="DRAM") as edram,
                tc.tile_pool(name="encp", bufs=6) as encp,
                tc.tile_pool(name="encnp", bufs=16) as encnp,
                tc.tile_pool(name="tanhp", bufs=3) as tanhp,
                tc.tile_pool(name="etp", bufs=2) as etp,
                tc.tile_pool(name="pbp", bufs=2) as pbp,
                tc.tile_pool(name="outp", bufs=2) as outp,
            ):
              def body(_iv=None):
                def scores_tile(b, t, tanh_t):
                    enct = encp.tile([128, 2 * NJ, 512], FP8)
                    nc.sync.dma_start(
                        out=enct,
                        in_=encdr[:, :, b * S + t * 512: b * S + (t + 1) * 512])
                    nil = 2 if (CT_ERED or CT_CTX) else 4
                    for dbp in range(NDB // nil):
                        pres = []
                        for h in range(nil):
                            pre_t = prep.tile([128, 512], F32, tag="pre",
                                              name=f"pre{h}")
                            pres.append(pre_t)
                        for j in range(NJ):      # bank-interleaved DR chains
                            for h in range(nil):
                                db = dbp * nil + h
                                nc.tensor.matmul(
                                    pres[h],
                                    w1e_sb[:, 2 * j:2 * j + 2, ts(db, 128)],
                                    enct[:, 2 * j:2 * j + 2, :],
                                    start=(j == 0), stop=(j == NJ - 1),
                                    perf_mode=DRW)
                        for h in range(nil):
                            db = dbp * nil + h
                            nc.scalar.activation(
                                tanh_t[:, db], pres[h], AF.Tanh,
                                bias=hv_sb[:, db * BC + b: db * BC + b + 1],
                                scale=SCALE)

                def ered_tile(t, tanh_t, eT):
                    # 4-way column-tiled: groups g=db%4 run concurrently in
                    # distinct 32-col PE groups (stationary zero-padded to
                    # M=32), partials land on partitions {0,32,64,96} of one
                    # PSUM bank (2 chained waves of 4).
                    e4_ps = epsp.tile([128, 512], F32)
                    if CT_ERED:
                        for db in range(NDB):
                            g = db % 4
                            nc.tensor.matmul(
                                e4_ps[32 * g:32 * g + 32, :],
                                w2_sb[:, db], tanh_t[:, db],
                                start=(db < 4), stop=(db >= 4),
                                tile_position=(0, 32 * g),
                                skip_group_check=True)
                    else:
                        for db in range(NDB):
                            nc.tensor.matmul(
                                e4_ps[0:1, :], w2_sb[:, db, 0:1], tanh_t[:, db],
                                start=(db == 0), stop=(db == NDB - 1))
                    e4_sb = smalls.tile([128, 512], F32, tag="esb")
                    if CT_ERED:
                        nc.vector.tensor_copy(e4_sb, e4_ps)
                    else:
                        nc.vector.tensor_copy(e4_sb[0:1, :], e4_ps[0:1, :])
                    # gather the partial rows, bounce through DRAM, read
                    # back transposed as [128 s, 4 c, 4 g], then g-sum on DVE
                    e_d = edram.tile([4, 512], F32)
                    es_ap = e4_sb[0:1, :]
                    if CT_ERED:
                        nc.sync.dma_start(
                            out=e_d,
                            in_=bass.AP(tensor=es_ap.tensor, offset=es_ap.offset,
                                        ap=[[32, 4]] + list(es_ap.ap[1:])))
                        eT4 = etp.tile([128, 4, 4], F32, tag="e4t")
                        ed_ap = e_d[0:1, :]
                        for g in range(4):
                            nc.sync.dma_start(
                                out=eT4[:, :, g],
                                in_=bass.AP(tensor=ed_ap.tensor,
                                            offset=ed_ap.offset + 512 * g,
                                            ap=[[1, 128], [128, 4]]))
                        s01 = smalls.tile([128, 4], F32, tag="s01")
                        nc.vector.tensor_add(s01, eT4[:, :, 0], eT4[:, :, 1])
                        s23 = smalls.tile([128, 4], F32, tag="s23")
                        nc.vector.tensor_add(s23, eT4[:, :, 2], eT4[:, :, 3])
                        nc.vector.tensor_add(eT[:, t * 4:(t + 1) * 4], s01, s23)
                    else:
                        nc.sync.dma_start(out=e_d[0:1, :], in_=e4_sb[0:1, :])
                        ed_ap = e_d[0:1, :]
                        nc.sync.dma_start(
                            out=eT[:, t * 4:(t + 1) * 4],
                            in_=bass.AP(tensor=ed_ap.tensor, offset=ed_ap.offset,
                                        ap=[[1, 128], [128, 4]]))

                def softmax(eT):
                    p_f = etp.tile([128, NST], F32, tag="pf")
                    pcs = smalls.tile([128, 1], F32, tag="pcs")
                    nc.scalar.activation(p_f, eT, AF.Exp, accum_out=pcs)
                    sum_ps = sumps.tile([1, 1], F32, tag="cfin")
                    nc.tensor.matmul(sum_ps, pcs, ones_col, start=True, stop=True)
                    rs = smalls.tile([1, 1], F32, tag="rs")
                    nc.vector.reciprocal(rs, sum_ps)
                    # p zero-padded to 32 stationary columns per s-tile
                    p32 = pbp.tile([128, NST, 32], BF16)
                    nc.vector.memset(p32, 0.0)
                    pc_ap = p32[:, :, 0:1]
                    nc.vector.tensor_copy(pc_ap, p_f)
                    return p32, rs

                def ctx_phase(b, p32, rs):
                    # 4-way column-tiled: s-tile st goes to group g=st%4
                    # (stationary zero-padded to M=32); the 4 partial ctx
                    # rows land on partitions {0,32,64,96}
                    ctx4_ps = miscps.tile([128, DK], F32, tag="misc")
                    for st in range(NST):
                        g = (st % 4) if CT_CTX else 0
                        encnt = encnp.tile([128, DK], BF16)
                        nc.sync.dma_start(
                            out=encnt,
                            in_=encn[b * S + st * 128: b * S + (st + 1) * 128, :])
                        for dh in range(2):
                            if CT_CTX:
                                nc.tensor.matmul(
                                    ctx4_ps[32 * g:32 * g + 32, ts(dh, 512)],
                                    p32[:, st], encnt[:, ts(dh, 512)],
                                    start=(st < 4), stop=(st >= NST - 4),
                                    tile_position=(0, 32 * g),
                                    skip_group_check=True)
                            else:
                                nc.tensor.matmul(
                                    ctx4_ps[0:1, ts(dh, 512)],
                                    p32[:, st, 0:1], encnt[:, ts(dh, 512)],
                                    start=(st == 0), stop=(st == NST - 1))
                    if CT_CTX:
                        ctx4_sb = smalls.tile([128, DK], F32, tag="c4sb")
                        nc.vector.tensor_copy(ctx4_sb, ctx4_ps)
                        ctxc = smalls.tile([4, DK], F32, tag="ctxc")
                        c4_ap = ctx4_sb[0:1, :]
                        nc.sync.dma_start(
                            out=ctxc,
                            in_=bass.AP(tensor=c4_ap.tensor, offset=c4_ap.offset,
                                        ap=[[32, 4]] + list(c4_ap.ap[1:])))
                        ctx_ps = sumps.tile([1, DK], F32, tag="cfin")
                        for dh in range(2):
                            nc.tensor.matmul(
                                ctx_ps[:, ts(dh, 512)], ones4,
                                ctxc[:, ts(dh, 512)], start=True, stop=True)
                        ctx_sb = outp.tile([1, DK], F32)
                        nc.vector.tensor_scalar_mul(ctx_sb, ctx_ps, rs)
                    else:
                        ctx_sb = outp.tile([1, DK], F32)
                        nc.vector.tensor_scalar_mul(ctx_sb, ctx4_ps[0:1, :], rs)
                    nc.sync.dma_start(out=out_d[b:b + 1, :], in_=ctx_sb)

                pend = None   # (last tanh tile, eT, b) of previous example
                prev = None   # (b, p32, rs) ready for ctx
                for b in range(BC):
                    eT = etp.tile([128, NST], F32, tag="eT")
                    tanh_tiles = []
                    for t in range(NT):
                        tanh_t = tanhp.tile([128, NDB, 512], BF16)
                        scores_tile(b, t, tanh_t)
                        tanh_tiles.append(tanh_t)
                        if t == 0 and pend is not None:
                            ered_tile(NT - 1, pend[0], pend[1])
                        if t == 1 and pend is not None:
                            p32, rs = softmax(pend[1])
                            prev = (pend[2], p32, rs)
                            pend = None
                        if t >= 1:
                            ered_tile(t - 1, tanh_tiles[t - 1], eT)
                        if t == NT - 1 and prev is not None:
                            ctx_phase(*prev)
                            prev = None
                    pend = (tanh_tiles[NT - 1], eT, b)
                ered_tile(NT - 1, pend[0], pend[1])
                p32, rs = softmax(pend[1])
                ctx_phase(pend[2], p32, rs)

              if reps == 1:
                  body()
              else:
                  with tc.For_i(0, reps, 1) as _i:
                      body(_i)

    nc.compile()
    return nc


def prep_inputs(hidden_state, encoder_outputs, W1, b1, W2):
    """Split + relayout + quantize full inputs into per-core in_maps."""
    E4, BF = ml_dtypes.float8_e4m3, ml_dtypes.bfloat16
    hidden_state = np.ascontiguousarray(hidden_state, dtype=np.float32)
    encoder_outputs = np.asarray(encoder_outputs, dtype=np.float32)
    W1 = np.asarray(W1, dtype=np.float32)
    b1 = np.asarray(b1, dtype=np.float32)
    W2 = np.asarray(W2, dtype=np.float32)

    W1e, W1h = W1[:, :DK], W1[:, DK:]
    # w1edr[p, 2j+i, d] = SW * W1e[d, j*256 + i*128 + p]
    w1edr = np.ascontiguousarray(
        (W1e.T * SW).reshape(NJ, 2, 128, D).transpose(2, 0, 1, 3)
        .reshape(128, 2 * NJ, D)).astype(E4)
    w1hT = np.ascontiguousarray(W1h.T.reshape(MC, 128, D).transpose(1, 0, 2))
    b1dp = b1.reshape(NDB, 128).T                     # [128, NDB]
    b1bc = np.ascontiguousarray(
        np.repeat(b1dp[:, :, None], BC, axis=2).reshape(128, NDB * BC),
        dtype=np.float32)
    w2dp = np.zeros((128, NDB, 32), dtype=BF)
    w2dp[:, :, 0] = W2.reshape(NDB, 128).T.astype(BF)

    in_maps = []
    for c in range(N_CORES):
        sl = slice(c * BC, (c + 1) * BC)
        enc_c = encoder_outputs[sl]                      # [BC, S, DK]
        # encdr[p, 2j+i, b*S+s] = SE * enc[b, s, j*256 + i*128 + p]
        encdr = np.ascontiguousarray(
            (enc_c.transpose(2, 0, 1).reshape(NJ, 2, 128, BC * S) * SE)
            .transpose(2, 0, 1, 3).reshape(128, 2 * NJ, BC * S)).astype(E4)
        encn = np.ascontiguousarray(enc_c.reshape(BC * S, DK)).astype(BF)
        h_c = hidden_state[sl]                           # [BC, D]
        hT = np.ascontiguousarray(h_c.T.reshape(MC, 128, BC).transpose(1, 0, 2))
        in_maps.append({
            "encdr": encdr, "encn": encn, "w1edr": w1edr,
            "w1hT": w1hT, "hT": hT, "b1bc": b1bc, "w2dp": w2dp,
        })
    return in_maps


_NC_CACHE = {}


def kernel(hidden_state, encoder_outputs, W1, b1, W2):
    if "nc" not in _NC_CACHE:
        _NC_CACHE["nc"] = build_kernel(reps=1)
    nc = _NC_CACHE["nc"]
    in_maps = prep_inputs(hidden_state, encoder_outputs, W1, b1, W2)
    res = run_bass_kernel_spmd(nc, in_maps, core_ids=list(range(N_CORES)))
    return np.concatenate([r["out"] for r in res.results], axis=0)


# revision 21
# speedup vs baseline: 4.0230x; 4.0230x over previous
"""Trainium2 Bass kernel for nn_Attention_update (additive attention pooling).

reference math (per example b):
    pre[s,d] = enc[b] @ W1e^T + (W1h @ h[b] + b1)      # [S, D]
    e[s]     = tanh(pre) @ W2[0]                        # [S]
    alpha    = softmax(e);  ctx = alpha @ enc[b]        # [DK]

Sharding: data-parallel over batch B=64 across 8 cores (8 examples/core),
same SPMD program on every core, no collectives.

Per-core kernel, fp8 DoubleRow score path in [d-part, s-free] layout:
  - scores: stationary = W1e chunks quantized to fp8e4 (pre-scaled x64 to
    dodge subnormal truncation), moving = enc in fp8e4 (pre-scaled x8),
    perf_mode=DoubleRow -> K=256 per instruction, ~2x bf16 MAC rate.
    pre lands as [128 d-part, 512 s-free] PSUM tiles (8 d-blocks x 4 j).
  - bias+tanh fused on ScalarE: tanh(pre/512 + hv[d]) with the per-example
    hv = W1h@h+b1 column as the ACT per-partition bias operand; output
    written bf16 straight into SBUF.
  - e[s] = sum_d W2_d tanh: K=128 matmul chain over the 8 d-blocks with
    W2 column stationary -> e as [1, 512] PSUM; a strided DMA transposes
    it into eT [128 s-part, 16]; ACT exp with accum_out gives p and its
    per-partition sum; ones-matmul + reciprocal finish the softmax scale.
  - ctx = sum_t p[:,t] (x) encN-tile chains in bf16, scaled by 1/sum.
    ctx of example b is emitted during the score phase of example b+1.
enc is supplied from host in fp8 (scores, 16.8MB) and bf16 (context,
33.5MB): ~50MB/core @ ~285GB/s hides under the PE-bound runtime.
Accuracy: fp8 quantization of enc/W1e gives rel err ~1e-2 on the fixed
harness inputs (vs 2e-2 gate); verified by numpy simulation + HW run.
"""
import os
import numpy as np
import ml_dtypes

import concourse.bass as bass
import concourse.mybir as mybir
import concourse.tile as tile
from concourse import bacc
from concourse.bass import ts
from concourse.bass_utils import run_bass_kernel_spmd

AF = mybir.ActivationFunctionType
ALU = mybir.AluOpType
F32 = mybir.dt.float32
F32R = mybir.dt.float32r
BF16 = mybir.dt.bfloat16
FP8 = mybir.dt.float8e4
DRW = mybir.MatmulPerfMode.DoubleRow

N_CORES = 8
B, S, DK, D = 64, 2048, 1024, 1024
BC = B // N_CORES          # examples per core
MC = D // 128              # m chunks (hidden dim)
NDB = D // 128             # d blocks
NJ = DK // 256             # DoubleRow k chunks (K=256 each)
NT = S // 512              # s512 tiles per example
NST = S // 128             # s tiles for ctx
SW = 64.0                  # W1e pre-quantization scale
SE = 8.0                   # enc pre-quantization scale
SCALE = 1.0 / (SW * SE)    # descale folded into ACT


CT_ERED = os.environ.get("CT_ERED", "0") == "1"
ABL = os.environ.get("ABL", "")
CT_CTX = os.environ.get("CT_CTX", "0") == "1"


def build_kernel(reps: int = 1):
    nc = bacc.Bacc(None)

    encdr = nc.dram_tensor("encdr", [128, 2 * NJ, BC * S], FP8, kind="ExternalInput")
    encn = nc.dram_tensor("encn", [BC * S, DK], BF16, kind="ExternalInput")
    w1edr = nc.dram_tensor("w1edr", [128, 2 * NJ, D], FP8, kind="ExternalInput")
    w1hT = nc.dram_tensor("w1hT", [128, MC, D], F32R, kind="ExternalInput")
    hT = nc.dram_tensor("hT", [128, MC, BC], F32R, kind="ExternalInput")
    b1bc = nc.dram_tensor("b1bc", [128, NDB * BC], F32, kind="ExternalInput")
    w2dp = nc.dram_tensor("w2dp", [128, NDB, 32], BF16, kind="ExternalInput")
    out_d = nc.dram_tensor("out", [BC, DK], F32, kind="ExternalOutput")

    with tile.TileContext(nc) as tc:
        with (
            tc.tile_pool(name="consts", bufs=1) as consts,
            tc.tile_pool(name="smalls", bufs=4) as smalls,
            tc.tile_pool(name="prep", bufs=3 if (CT_ERED or CT_CTX) else 4,
                         space="PSUM") as prep,
            tc.tile_pool(name="epsp", bufs=1, space="PSUM") as epsp,
            tc.tile_pool(name="miscps", bufs=1, space="PSUM") as miscps,
            tc.tile_pool(name="sumps", bufs=1, space="PSUM") as sumps,
        ):
            # ---- constants / parameters ----
            w1e_sb = consts.tile([128, 2 * NJ, D], FP8)
            nc.sync.dma_start(out=w1e_sb, in_=w1edr[:, :, :])
            w2_sb = consts.tile([128, NDB, 32], BF16)
            nc.sync.dma_start(out=w2_sb, in_=w2dp[:, :, :])
            ones_col = consts.tile([128, 1], F32)
            nc.vector.memset(ones_col, 1.0)
            ones4 = consts.tile([4, 1], F32)
            nc.vector.memset(ones4, 1.0)
            hv_sb = consts.tile([128, NDB * BC], F32)
            ones16 = consts.tile([128, NST], F32)
            nc.vector.memset(ones16, 1.0)

            # ---- startup: hv[d, b] = (W1h @ h + b1) in d-part layout ----
            with tc.tile_pool(name="w1hp", bufs=1) as w1hp:
                w1hT_sb = w1hp.tile([128, MC, D], F32R)
                nc.sync.dma_start(out=w1hT_sb, in_=w1hT[:, :, :])
                hT_sb = w1hp.tile([128, MC, BC], F32R)
                nc.sync.dma_start(out=hT_sb, in_=hT[:, :, :])
                b1_sb = w1hp.tile([128, NDB * BC], F32)
                nc.sync.dma_start(out=b1_sb, in_=b1bc[:, :])
                hv_ps = miscps.tile([128, NDB * BC], F32, tag="misc")
                for db in range(NDB):
                    sl = slice(db * BC, (db + 1) * BC)
                    for mc in range(MC):
                        nc.tensor.matmul(
                            hv_ps[:, sl], w1hT_sb[:, mc, ts(db, 128)],
                            hT_sb[:, mc, :], start=(mc == 0), stop=(mc == MC - 1))
                nc.vector.tensor_add(hv_sb, hv_ps, b1_sb)

            with (
                tc.tile_pool(name="edram", bufs=2, space="DRAM") as edram,
                tc.tile_pool(name="encp", bufs=6) as encp,
                tc.tile_pool(name="encnp", bufs=16) as encnp,
                tc.tile_pool(name="tanhp", bufs=5) as tanhp,
                tc.tile_pool(name="etp", bufs=2) as etp,
                tc.tile_pool(name="pbp", bufs=2) as pbp,
                tc.tile_pool(name="outp", bufs=2) as outp,
            ):
              def body(_iv=None):
                def scores_pair(b, tp, tanhs):
                    # two s512 tiles share each stationary load: per (db, j)
                    # the 213ns DR LDWEIGHTS hides under 2x512-col streams
                    encts = []
                    for h in range(2):
                        t = 2 * tp + h
                        enct = encp.tile([128, 2 * NJ, 512], FP8,
                                         name=f"enct{h}")
                        nc.sync.dma_start(
                            out=enct,
                            in_=encdr[:, :, b * S + t * 512:
                                      b * S + (t + 1) * 512])
                        encts.append(enct)
                    for db in range(NDB):
                        pres = []
                        for h in range(2):
                            pre_t = prep.tile([128, 512], F32, tag="pre",
                                              name=f"pre{h}")
                            pres.append(pre_t)
                        for j in range(NJ):
                            for h in range(2):
                                nc.tensor.matmul(
                                    pres[h],
                                    w1e_sb[:, 2 * j:2 * j + 2, ts(db, 128)],
                                    encts[h][:, 2 * j:2 * j + 2, :],
                                    start=(j == 0), stop=(j == NJ - 1),
                                    perf_mode=DRW)
                        for h in range(2):
                            nc.scalar.activation(
                                tanhs[h][:, db], pres[h], AF.Tanh,
                                bias=hv_sb[:, db * BC + b: db * BC + b + 1],
                                scale=SCALE)

                def ered_tile(t, tanh_t, eT):
                    # 4-way column-tiled: groups g=db%4 run concurrently in
                    # distinct 32-col PE groups (stationary zero-padded to
                    # M=32), partials land on partitions {0,32,64,96} of one
                    # PSUM bank (2 chained waves of 4).
                    e4_ps = epsp.tile([128, 512], F32)
                    if CT_ERED:
                        for db in range(NDB):
                            g = db % 4
                            nc.tensor.matmul(
                                e4_ps[32 * g:32 * g + 32, :],
                                w2_sb[:, db], tanh_t[:, db],
                                start=(db < 4), stop=(db >= 4),
                                tile_position=(0, 32 * g),
                                skip_group_check=True)
                    else:
                        for db in range(NDB):
                            nc.tensor.matmul(
                                e4_ps[0:1, :], w2_sb[:, db, 0:1], tanh_t[:, db],
                                start=(db == 0), stop=(db == NDB - 1))
                    e4_sb = smalls.tile([128, 512], F32, tag="esb")
                    if CT_ERED:
                        nc.vector.tensor_copy(e4_sb, e4_ps)
                    else:
                        nc.vector.tensor_copy(e4_sb[0:1, :], e4_ps[0:1, :])
                    # gather the partial rows, bounce through DRAM, read
                    # back transposed as [128 s, 4 c, 4 g], then g-sum on DVE
                    e_d = edram.tile([4, 512], F32)
                    es_ap = e4_sb[0:1, :]
                    if CT_ERED:
                        nc.sync.dma_start(
                            out=e_d,
                            in_=bass.AP(tensor=es_ap.tensor, offset=es_ap.offset,
                                        ap=[[32, 4]] + list(es_ap.ap[1:])))
                        eT4 = etp.tile([128, 4, 4], F32, tag="e4t")
                        ed_ap = e_d[0:1, :]
                        for g in range(4):
                            nc.sync.dma_start(
                                out=eT4[:, :, g],
                                in_=bass.AP(tensor=ed_ap.tensor,
                                            offset=ed_ap.offset + 512 * g,
                                            ap=[[1, 128], [128, 4]]))
                        s01 = smalls.tile([128, 4], F32, tag="s01")
                        nc.vector.tensor_add(s01, eT4[:, :, 0], eT4[:, :, 1])
                        s23 = smalls.tile([128, 4], F32, tag="s23")
                        nc.vector.tensor_add(s23, eT4[:, :, 2], eT4[:, :, 3])
                        nc.vector.tensor_add(eT[:, t * 4:(t + 1) * 4], s01, s23)
                    else:
                        nc.sync.dma_start(out=e_d[0:1, :], in_=e4_sb[0:1, :])
                        ed_ap = e_d[0:1, :]
                        nc.sync.dma_start(
                            out=eT[:, t * 4:(t + 1) * 4],
                            in_=bass.AP(tensor=ed_ap.tensor, offset=ed_ap.offset,
                                        ap=[[1, 128], [128, 4]]))

                def softmax(eT):
                    # exp via tanh identity (avoids ACT table reloads:
                    # exp and tanh live in different act-table sets):
                    # e^x = (1 + tanh(x/2)) / (1 - tanh(x/2))
                    th = etp.tile([128, NST], F32, tag="th")
                    nc.scalar.activation(th, eT, AF.Tanh, scale=0.5)
                    num = smalls.tile([128, NST], F32, tag="num")
                    nc.vector.tensor_add(num, ones16, th)
                    den = smalls.tile([128, NST], F32, tag="den")
                    nc.vector.scalar_tensor_tensor(
                        out=den, in0=th, scalar=-1.0, in1=ones16,
                        op0=ALU.mult, op1=ALU.add)
                    rden = smalls.tile([128, NST], F32, tag="rden")
                    nc.vector.reciprocal(rden, den)
                    p_f = etp.tile([128, NST], F32, tag="pf")
                    nc.vector.scalar_tensor_tensor(
                        out=p_f, in0=num, scalar=0.0, in1=rden,
                        op0=ALU.add, op1=ALU.mult)
                    pcs = smalls.tile([128, 1], F32, tag="pcs")
                    nc.vector.reduce_sum(pcs, p_f, axis=mybir.AxisListType.X)
                    sum_ps = sumps.tile([1, 1], F32, tag="cfin")
                    nc.tensor.matmul(sum_ps, pcs, ones_col, start=True, stop=True)
                    rs = smalls.tile([1, 1], F32, tag="rs")
                    nc.vector.reciprocal(rs, sum_ps)
                    # p zero-padded to 32 stationary columns per s-tile
                    p32 = pbp.tile([128, NST, 32], BF16)
                    nc.vector.memset(p32, 0.0)
                    pc_ap = p32[:, :, 0:1]
                    nc.vector.tensor_copy(pc_ap, p_f)
                    return p32, rs

                def ctx_phase(b, p32, rs):
                    # 4-way column-tiled: s-tile st goes to group g=st%4
                    # (stationary zero-padded to M=32); the 4 partial ctx
                    # rows land on partitions {0,32,64,96}
                    ctx4_ps = miscps.tile([128, DK], F32, tag="misc")
                    for st in range(NST):
                        g = (st % 4) if CT_CTX else 0
                        encnt = encnp.tile([128, DK], BF16)
                        nc.sync.dma_start(
                            out=encnt,
                            in_=encn[b * S + st * 128: b * S + (st + 1) * 128, :])
                        for dh in range(2):
                            if CT_CTX:
                                nc.tensor.matmul(
                                    ctx4_ps[32 * g:32 * g + 32, ts(dh, 512)],
                                    p32[:, st], encnt[:, ts(dh, 512)],
                                    start=(st < 4), stop=(st >= NST - 4),
                                    tile_position=(0, 32 * g),
                                    skip_group_check=True)
                            else:
                                nc.tensor.matmul(
                                    ctx4_ps[0:1, ts(dh, 512)],
                                    p32[:, st, 0:1], encnt[:, ts(dh, 512)],
                                    start=(st == 0), stop=(st == NST - 1))
                    if CT_CTX:
                        ctx4_sb = smalls.tile([128, DK], F32, tag="c4sb")
                        nc.vector.tensor_copy(ctx4_sb, ctx4_ps)
                        ctxc = smalls.tile([4, DK], F32, tag="ctxc")
                        c4_ap = ctx4_sb[0:1, :]
                        nc.sync.dma_start(
                            out=ctxc,
                            in_=bass.AP(tensor=c4_ap.tensor, offset=c4_ap.offset,
                                        ap=[[32, 4]] + list(c4_ap.ap[1:])))
                        ctx_ps = sumps.tile([1, DK], F32, tag="cfin")
                        for dh in range(2):
                            nc.tensor.matmul(
                                ctx_ps[:, ts(dh, 512)], ones4,
                                ctxc[:, ts(dh, 512)], start=True, stop=True)
                        ctx_sb = outp.tile([1, DK], F32)
                        nc.vector.tensor_scalar_mul(ctx_sb, ctx_ps, rs)
                    else:
                        ctx_sb = outp.tile([1, DK], F32)
                        nc.vector.tensor_scalar_mul(ctx_sb, ctx4_ps[0:1, :], rs)
                    nc.sync.dma_start(out=out_d[b:b + 1, :], in_=ctx_sb)

                pend = None   # (last tanh tile, eT, b) of previous example
                prev = None   # (b, p32, rs) ready for ctx
                for b in range(BC):
                    eT = etp.tile([128, NST], F32, tag="eT")
                    tts = []
                    for tp in range(2):
                        tanhs = []
                        for h in range(2):
                            ta = tanhp.tile([128, NDB, 512], BF16,
                                            name=f"ta{h}")
                            tanhs.append(ta)
                        scores_pair(b, tp, tanhs)
                        tts += tanhs
                        if ABL == "scores":
                            continue
                        if tp == 0:
                            if pend is not None:
                                ered_tile(NT - 1, pend[0], pend[1])
                                p32, rs = softmax(pend[1])
                                prev = (pend[2], p32, rs)
                                pend = None
                        else:
                            ered_tile(0, tts[0], eT)
                            ered_tile(1, tts[1], eT)
                            if prev is not None:
                                ctx_phase(*prev)
                                prev = None
                    if ABL == "scores":
                        continue
                    ered_tile(2, tts[2], eT)
                    pend = (tts[3], eT, b)
                if ABL != "scores":
                    ered_tile(NT - 1, pend[0], pend[1])
                    p32, rs = softmax(pend[1])
                    ctx_phase(pend[2], p32, rs)
                else:
                    zt = outp.tile([1, DK], F32, name="zt")
                    nc.vector.tensor_copy(zt[0:1, 0:512], tts[3][0:1, 0, :])
                    nc.sync.dma_start(out=out_d[0:1, :], in_=zt)

              if reps == 1:
                  body()
              else:
                  with tc.For_i(0, reps, 1) as _i:
                      body(_i)

    nc.compile()
    return nc


def prep_inputs(hidden_state, encoder_outputs, W1, b1, W2):
    """Split + relayout + quantize full inputs into per-core in_maps."""
    E4, BF = ml_dtypes.float8_e4m3, ml_dtypes.bfloat16
    hidden_state = np.ascontiguousarray(hidden_state, dtype=np.float32)
    encoder_outputs = np.asarray(encoder_outputs, dtype=np.float32)
    W1 = np.asarray(W1, dtype=np.float32)
    b1 = np.asarray(b1, dtype=np.float32)
    W2 = np.asarray(W2, dtype=np.float32)

    W1e, W1h = W1[:, :DK], W1[:, DK:]
    # w1edr[p, 2j+i, d] = SW * W1e[d, j*256 + i*128 + p]
    w1edr = np.ascontiguousarray(
        (W1e.T * SW).reshape(NJ, 2, 128, D).transpose(2, 0, 1, 3)
        .reshape(128, 2 * NJ, D)).astype(E4)
    w1hT = np.ascontiguousarray(W1h.T.reshape(MC, 128, D).transpose(1, 0, 2))
    b1dp = b1.reshape(NDB, 128).T                     # [128, NDB]
    b1bc = np.ascontiguousarray(
        np.repeat(b1dp[:, :, None], BC, axis=2).reshape(128, NDB * BC),
        dtype=np.float32)
    w2dp = np.zeros((128, NDB, 32), dtype=BF)
    w2dp[:, :, 0] = W2.reshape(NDB, 128).T.astype(BF)

    in_maps = []
    for c in range(N_CORES):
        sl = slice(c * BC, (c + 1) * BC)
        enc_c = encoder_outputs[sl]                      # [BC, S, DK]
        # encdr[p, 2j+i, b*S+s] = SE * enc[b, s, j*256 + i*128 + p]
        encdr = np.ascontiguousarray(
            (enc_c.transpose(2, 0, 1).reshape(NJ, 2, 128, BC * S) * SE)
            .transpose(2, 0, 1, 3).reshape(128, 2 * NJ, BC * S)).astype(E4)
        encn = np.ascontiguousarray(enc_c.reshape(BC * S, DK)).astype(BF)
        h_c = hidden_state[sl]                           # [BC, D]
        hT = np.ascontiguousarray(h_c.T.reshape(MC, 128, BC).transpose(1, 0, 2))
        in_maps.append({
            "encdr": encdr, "encn": encn, "w1edr": w1edr,
            "w1hT": w1hT, "hT": hT, "b1bc": b1bc, "w2dp": w2dp,
        })
    return in_maps


_NC_CACHE = {}


def kernel(hidden_state, encoder_outputs, W1, b1, W2):
    if "nc" not in _NC_CACHE:
        _NC_CACHE["nc"] = build_kernel(reps=1)
    nc = _NC_CACHE["nc"]
    in_maps = prep_inputs(hidden_state, encoder_outputs, W1, b1, W2)
    res = run_bass_kernel_spmd(nc, in_maps, core_ids=list(range(N_CORES)))
    return np.concatenate([r["out"] for r in res.results], axis=0)


# revision 22
# speedup vs baseline: 5.7270x; 1.4236x over previous
"""Trainium2 Bass kernel for nn_Attention_update (additive attention pooling).

reference math (per example b):
    pre[s,d] = enc[b] @ W1e^T + (W1h @ h[b] + b1)      # [S, D]
    e[s]     = tanh(pre) @ W2[0]                        # [S]
    alpha    = softmax(e);  ctx = alpha @ enc[b]        # [DK]

Sharding: data-parallel over batch B=64 across 8 cores (8 examples/core),
same SPMD program on every core, no collectives.

Per-core kernel, fp8 DoubleRow score path in [d-part, s-free] layout:
  - scores: stationary = W1e chunks quantized to fp8e4 (pre-scaled x64 to
    dodge subnormal truncation), moving = enc in fp8e4 (pre-scaled x8),
    perf_mode=DoubleRow -> K=256 per instruction, ~2x bf16 MAC rate.
    pre lands as [128 d-part, 512 s-free] PSUM tiles (8 d-blocks x 4 j).
  - bias+tanh fused on ScalarE: tanh(pre/512 + hv[d]) with the per-example
    hv = W1h@h+b1 column as the ACT per-partition bias operand; output
    written bf16 straight into SBUF.
  - e[s] = sum_d W2_d tanh: K=128 matmul chain over the 8 d-blocks with
    W2 column stationary -> e as [1, 512] PSUM; a strided DMA transposes
    it into eT [128 s-part, 16]; ACT exp with accum_out gives p and its
    per-partition sum; ones-matmul + reciprocal finish the softmax scale.
  - ctx = sum_t p[:,t] (x) encN-tile chains in bf16, scaled by 1/sum.
    ctx of example b is emitted during the score phase of example b+1.
enc is supplied from host in fp8 (scores, 16.8MB) and bf16 (context,
33.5MB): ~50MB/core @ ~285GB/s hides under the PE-bound runtime.
Accuracy: fp8 quantization of enc/W1e gives rel err ~1e-2 on the fixed
harness inputs (vs 2e-2 gate); verified by numpy simulation + HW run.
"""
import os
import numpy as np
import ml_dtypes

import concourse.bass as bass
import concourse.mybir as mybir
import concourse.tile as tile
from concourse import bacc
from concourse.bass import ts
from concourse.bass_utils import run_bass_kernel_spmd

AF = mybir.ActivationFunctionType
ALU = mybir.AluOpType
F32 = mybir.dt.float32
F32R = mybir.dt.float32r
BF16 = mybir.dt.bfloat16
FP8 = mybir.dt.float8e4
DRW = mybir.MatmulPerfMode.DoubleRow

N_CORES = 8
B, S, DK, D = 64, 2048, 1024, 1024
BC = B // N_CORES          # examples per core
MC = D // 128              # m chunks (hidden dim)
NDB = D // 128             # d blocks
NJ = DK // 256             # DoubleRow k chunks (K=256 each)
NT = S // 512              # s512 tiles per example
NST = S // 128             # s tiles for ctx
SW = 64.0                  # W1e pre-quantization scale
SE = 8.0                   # enc pre-quantization scale
SCALE = 1.0 / (SW * SE)    # descale folded into ACT


CT_ERED = os.environ.get("CT_ERED", "0") == "1"
ABL = os.environ.get("ABL", "")
CT_CTX = os.environ.get("CT_CTX", "0") == "1"


def build_kernel(reps: int = 1):
    nc = bacc.Bacc(None)

    encdr = nc.dram_tensor("encdr", [128, 2 * NJ, BC * S], FP8, kind="ExternalInput")
    encn = nc.dram_tensor("encn", [BC * S, DK], BF16, kind="ExternalInput")
    w1edr = nc.dram_tensor("w1edr", [128, 2 * NJ, D], FP8, kind="ExternalInput")
    w1hT = nc.dram_tensor("w1hT", [128, MC, D], F32R, kind="ExternalInput")
    hT = nc.dram_tensor("hT", [128, MC, BC], F32R, kind="ExternalInput")
    b1bc = nc.dram_tensor("b1bc", [128, NDB * BC], F32, kind="ExternalInput")
    w2dp = nc.dram_tensor("w2dp", [128, NDB, 32], BF16, kind="ExternalInput")
    out_d = nc.dram_tensor("out", [BC, DK], F32, kind="ExternalOutput")

    with tile.TileContext(nc) as tc:
        with (
            tc.tile_pool(name="consts", bufs=1) as consts,
            tc.tile_pool(name="smalls", bufs=4) as smalls,
            tc.tile_pool(name="prep", bufs=3 if (CT_ERED or CT_CTX) else 4,
                         space="PSUM") as prep,
            tc.tile_pool(name="epsp", bufs=1, space="PSUM") as epsp,
            tc.tile_pool(name="miscps", bufs=1, space="PSUM") as miscps,
            tc.tile_pool(name="sumps", bufs=1, space="PSUM") as sumps,
        ):
            # ---- constants / parameters ----
            w1e_sb = consts.tile([128, 2 * NJ, D], FP8)
            nc.sync.dma_start(out=w1e_sb, in_=w1edr[:, :, :])
            w2_sb = consts.tile([128, NDB, 32], BF16)
            nc.sync.dma_start(out=w2_sb, in_=w2dp[:, :, :])
            ones_col = consts.tile([128, 1], F32)
            nc.vector.memset(ones_col, 1.0)
            ones4 = consts.tile([4, 1], F32)
            nc.vector.memset(ones4, 1.0)
            hv_sb = consts.tile([128, NDB * BC], F32)
            ones16 = consts.tile([128, NST], F32)
            nc.vector.memset(ones16, 1.0)

            # ---- startup: hv[d, b] = (W1h @ h + b1) in d-part layout ----
            with tc.tile_pool(name="w1hp", bufs=1) as w1hp:
                w1hT_sb = w1hp.tile([128, MC, D], F32R)
                nc.sync.dma_start(out=w1hT_sb, in_=w1hT[:, :, :])
                hT_sb = w1hp.tile([128, MC, BC], F32R)
                nc.sync.dma_start(out=hT_sb, in_=hT[:, :, :])
                b1_sb = w1hp.tile([128, NDB * BC], F32)
                nc.sync.dma_start(out=b1_sb, in_=b1bc[:, :])
                hv_ps = miscps.tile([128, NDB * BC], F32, tag="misc")
                for db in range(NDB):
                    sl = slice(db * BC, (db + 1) * BC)
                    for mc in range(MC):
                        nc.tensor.matmul(
                            hv_ps[:, sl], w1hT_sb[:, mc, ts(db, 128)],
                            hT_sb[:, mc, :], start=(mc == 0), stop=(mc == MC - 1))
                nc.vector.tensor_add(hv_sb, hv_ps, b1_sb)

            with (
                tc.tile_pool(name="edram", bufs=2, space="DRAM") as edram,
                tc.tile_pool(name="encp", bufs=6) as encp,
                tc.tile_pool(name="encnp", bufs=16) as encnp,
                tc.tile_pool(name="tanhp", bufs=5) as tanhp,
                tc.tile_pool(name="etp", bufs=2) as etp,
                tc.tile_pool(name="pbp", bufs=2) as pbp,
                tc.tile_pool(name="outp", bufs=2) as outp,
            ):
              def body(_iv=None):
                def scores_pair(b, tp, tanhs):
                    # two s512 tiles share each stationary load: per (db, j)
                    # the 213ns DR LDWEIGHTS hides under 2x512-col streams
                    encts = []
                    for h in range(2):
                        t = 2 * tp + h
                        enct = encp.tile([128, 2 * NJ, 512], FP8,
                                         name=f"enct{h}")
                        nc.sync.dma_start(
                            out=enct,
                            in_=encdr[:, :, b * S + t * 512:
                                      b * S + (t + 1) * 512])
                        encts.append(enct)
                    for db in range(NDB):
                        pres = []
                        for h in range(2):
                            pre_t = prep.tile([128, 512], F32, tag="pre",
                                              name=f"pre{h}")
                            pres.append(pre_t)
                        for j in range(NJ):
                            for h in range(2):
                                nc.tensor.matmul(
                                    pres[h],
                                    w1e_sb[:, 2 * j:2 * j + 2, ts(db, 128)],
                                    encts[h][:, 2 * j:2 * j + 2, :],
                                    start=(j == 0), stop=(j == NJ - 1),
                                    perf_mode=DRW)
                        for h in range(2):
                            nc.scalar.activation(
                                tanhs[h][:, db], pres[h], AF.Tanh,
                                bias=hv_sb[:, db * BC + b: db * BC + b + 1],
                                scale=SCALE)

                def ered_tile(t, tanh_t, eT):
                    # 4-way column-tiled: groups g=db%4 run concurrently in
                    # distinct 32-col PE groups (stationary zero-padded to
                    # M=32), partials land on partitions {0,32,64,96} of one
                    # PSUM bank (2 chained waves of 4).
                    e4_ps = epsp.tile([128, 512], F32)
                    if CT_ERED:
                        for db in range(NDB):
                            g = db % 4
                            nc.tensor.matmul(
                                e4_ps[32 * g:32 * g + 32, :],
                                w2_sb[:, db], tanh_t[:, db],
                                start=(db < 4), stop=(db >= 4),
                                tile_position=(0, 32 * g),
                                skip_group_check=True)
                    else:
                        for db in range(NDB):
                            nc.tensor.matmul(
                                e4_ps[0:1, :], w2_sb[:, db, 0:1], tanh_t[:, db],
                                start=(db == 0), stop=(db == NDB - 1))
                    e4_sb = smalls.tile([128, 512], F32, tag="esb")
                    if CT_ERED:
                        nc.vector.tensor_copy(e4_sb, e4_ps)
                    else:
                        nc.vector.tensor_copy(e4_sb[0:1, :], e4_ps[0:1, :])
                    # gather the partial rows, bounce through DRAM, read
                    # back transposed as [128 s, 4 c, 4 g], then g-sum on DVE
                    e_d = edram.tile([4, 512], F32)
                    es_ap = e4_sb[0:1, :]
                    if CT_ERED:
                        nc.sync.dma_start(
                            out=e_d,
                            in_=bass.AP(tensor=es_ap.tensor, offset=es_ap.offset,
                                        ap=[[32, 4]] + list(es_ap.ap[1:])))
                        eT4 = etp.tile([128, 4, 4], F32, tag="e4t")
                        ed_ap = e_d[0:1, :]
                        for g in range(4):
                            nc.sync.dma_start(
                                out=eT4[:, :, g],
                                in_=bass.AP(tensor=ed_ap.tensor,
                                            offset=ed_ap.offset + 512 * g,
                                            ap=[[1, 128], [128, 4]]))
                        s01 = smalls.tile([128, 4], F32, tag="s01")
                        nc.vector.tensor_add(s01, eT4[:, :, 0], eT4[:, :, 1])
                        s23 = smalls.tile([128, 4], F32, tag="s23")
                        nc.vector.tensor_add(s23, eT4[:, :, 2], eT4[:, :, 3])
                        nc.vector.tensor_add(eT[:, t * 4:(t + 1) * 4], s01, s23)
                    else:
                        nc.sync.dma_start(out=e_d[0:1, :], in_=e4_sb[0:1, :])
                        ed_ap = e_d[0:1, :]
                        nc.sync.dma_start(
                            out=eT[:, t * 4:(t + 1) * 4],
                            in_=bass.AP(tensor=ed_ap.tensor, offset=ed_ap.offset,
                                        ap=[[1, 128], [128, 4]]))

                def softmax(eT):
                    # exp via tanh identity (avoids ACT table reloads:
                    # exp and tanh live in different act-table sets):
                    # e^x = (1 + tanh(x/2)) / (1 - tanh(x/2))
                    th = etp.tile([128, NST], F32, tag="th")
                    nc.scalar.activation(th, eT, AF.Tanh, scale=0.5)
                    num = smalls.tile([128, NST], F32, tag="num")
                    nc.vector.tensor_add(num, ones16, th)
                    den = smalls.tile([128, NST], F32, tag="den")
                    nc.vector.scalar_tensor_tensor(
                        out=den, in0=th, scalar=-1.0, in1=ones16,
                        op0=ALU.mult, op1=ALU.add)
                    rden = smalls.tile([128, NST], F32, tag="rden")
                    nc.vector.reciprocal(rden, den)
                    p_f = etp.tile([128, NST], F32, tag="pf")
                    nc.vector.scalar_tensor_tensor(
                        out=p_f, in0=num, scalar=0.0, in1=rden,
                        op0=ALU.add, op1=ALU.mult)
                    pcs = smalls.tile([128, 1], F32, tag="pcs")
                    nc.vector.reduce_sum(pcs, p_f, axis=mybir.AxisListType.X)
                    sum_ps = sumps.tile([1, 1], F32, tag="cfin")
                    nc.tensor.matmul(sum_ps, pcs, ones_col, start=True, stop=True)
                    rs = smalls.tile([1, 1], F32, tag="rs")
                    nc.vector.reciprocal(rs, sum_ps)
                    # p zero-padded to 32 stationary columns per s-tile
                    p32 = pbp.tile([128, NST, 32], BF16)
                    nc.vector.memset(p32, 0.0)
                    pc_ap = p32[:, :, 0:1]
                    nc.vector.tensor_copy(pc_ap, p_f)
                    return p32, rs

                def ctx_prefetch(b, st0, st1):
                    tiles = []
                    for st in range(st0, st1):
                        encnt = encnp.tile([128, DK], BF16, name="encnt")
                        nc.sync.dma_start(
                            out=encnt,
                            in_=encn[b * S + st * 128: b * S + (st + 1) * 128, :])
                        tiles.append(encnt)
                    return tiles

                def ctx_phase(b, p32, rs, encnts):
                    # 4-way column-tiled: s-tile st goes to group g=st%4
                    # (stationary zero-padded to M=32); the 4 partial ctx
                    # rows land on partitions {0,32,64,96}
                    ctx4_ps = miscps.tile([128, DK], F32, tag="misc")
                    for st in range(NST):
                        g = (st % 4) if CT_CTX else 0
                        encnt = encnts[st]
                        for dh in range(2):
                            if CT_CTX:
                                nc.tensor.matmul(
                                    ctx4_ps[32 * g:32 * g + 32, ts(dh, 512)],
                                    p32[:, st], encnt[:, ts(dh, 512)],
                                    start=(st < 4), stop=(st >= NST - 4),
                                    tile_position=(0, 32 * g),
                                    skip_group_check=True)
                            else:
                                nc.tensor.matmul(
                                    ctx4_ps[0:1, ts(dh, 512)],
                                    p32[:, st, 0:1], encnt[:, ts(dh, 512)],
                                    start=(st == 0), stop=(st == NST - 1))
                    if CT_CTX:
                        ctx4_sb = smalls.tile([128, DK], F32, tag="c4sb")
                        nc.vector.tensor_copy(ctx4_sb, ctx4_ps)
                        ctxc = smalls.tile([4, DK], F32, tag="ctxc")
                        c4_ap = ctx4_sb[0:1, :]
                        nc.sync.dma_start(
                            out=ctxc,
                            in_=bass.AP(tensor=c4_ap.tensor, offset=c4_ap.offset,
                                        ap=[[32, 4]] + list(c4_ap.ap[1:])))
                        ctx_ps = sumps.tile([1, DK], F32, tag="cfin")
                        for dh in range(2):
                            nc.tensor.matmul(
                                ctx_ps[:, ts(dh, 512)], ones4,
                                ctxc[:, ts(dh, 512)], start=True, stop=True)
                        ctx_sb = outp.tile([1, DK], F32)
                        nc.vector.tensor_scalar_mul(ctx_sb, ctx_ps, rs)
                    else:
                        ctx_sb = outp.tile([1, DK], F32)
                        nc.vector.tensor_scalar_mul(ctx_sb, ctx4_ps[0:1, :], rs)
                    nc.sync.dma_start(out=out_d[b:b + 1, :], in_=ctx_sb)

                pend = None   # (last tanh tile, eT, b) of previous example
                prev = None   # (b, p32, rs) ready for ctx
                enc_pf = []
                for b in range(BC):
                    eT = etp.tile([128, NST], F32, tag="eT")
                    tts = []
                    for tp in range(2):
                        tanhs = []
                        for h in range(2):
                            ta = tanhp.tile([128, NDB, 512], BF16,
                                            name=f"ta{h}")
                            tanhs.append(ta)
                        scores_pair(b, tp, tanhs)
                        tts += tanhs
                        if ABL == "scores":
                            continue
                        if tp == 0:
                            if pend is not None:
                                ered_tile(NT - 1, pend[0], pend[1])
                                p32, rs = softmax(pend[1])
                                prev = (pend[2], p32, rs)
                                enc_pf = ctx_prefetch(prev[0], 0, NST // 2)
                                pend = None
                        else:
                            if prev is not None:
                                enc_pf += ctx_prefetch(prev[0], NST // 2, NST)
                            ered_tile(0, tts[0], eT)
                            ered_tile(1, tts[1], eT)
                            if prev is not None:
                                ctx_phase(*prev, enc_pf)
                                prev = None
                    if ABL == "scores":
                        continue
                    ered_tile(2, tts[2], eT)
                    pend = (tts[3], eT, b)
                if ABL != "scores":
                    ered_tile(NT - 1, pend[0], pend[1])
                    p32, rs = softmax(pend[1])
                    enc_pf = ctx_prefetch(pend[2], 0, NST)
                    ctx_phase(pend[2], p32, rs, enc_pf)
                else:
                    zt = outp.tile([1, DK], F32, name="zt")
                    nc.vector.tensor_copy(zt[0:1, 0:512], tts[3][0:1, 0, :])
                    nc.sync.dma_start(out=out_d[0:1, :], in_=zt)

              if reps == 1:
                  body()
              else:
                  with tc.For_i(0, reps, 1) as _i:
                      body(_i)

    nc.compile()
    return nc


def prep_inputs(hidden_state, encoder_outputs, W1, b1, W2):
    """Split + relayout + quantize full inputs into per-core in_maps."""
    E4, BF = ml_dtypes.float8_e4m3, ml_dtypes.bfloat16
    hidden_state = np.ascontiguousarray(hidden_state, dtype=np.float32)
    encoder_outputs = np.asarray(encoder_outputs, dtype=np.float32)
    W1 = np.asarray(W1, dtype=np.float32)
    b1 = np.asarray(b1, dtype=np.float32)
    W2 = np.asarray(W2, dtype=np.float32)

    W1e, W1h = W1[:, :DK], W1[:, DK:]
    # w1edr[p, 2j+i, d] = SW * W1e[d, j*256 + i*128 + p]
    w1edr = np.ascontiguousarray(
        (W1e.T * SW).reshape(NJ, 2, 128, D).transpose(2, 0, 1, 3)
        .reshape(128, 2 * NJ, D)).astype(E4)
    w1hT = np.ascontiguousarray(W1h.T.reshape(MC, 128, D).transpose(1, 0, 2))
    b1dp = b1.reshape(NDB, 128).T                     # [128, NDB]
    b1bc = np.ascontiguousarray(
        np.repeat(b1dp[:, :, None], BC, axis=2).reshape(128, NDB * BC),
        dtype=np.float32)
    w2dp = np.zeros((128, NDB, 32), dtype=BF)
    w2dp[:, :, 0] = W2.reshape(NDB, 128).T.astype(BF)

    in_maps = []
    for c in range(N_CORES):
        sl = slice(c * BC, (c + 1) * BC)
        enc_c = encoder_outputs[sl]                      # [BC, S, DK]
        # encdr[p, 2j+i, b*S+s] = SE * enc[b, s, j*256 + i*128 + p]
        encdr = np.ascontiguousarray(
            (enc_c.transpose(2, 0, 1).reshape(NJ, 2, 128, BC * S) * SE)
            .transpose(2, 0, 1, 3).reshape(128, 2 * NJ, BC * S)).astype(E4)
        encn = np.ascontiguousarray(enc_c.reshape(BC * S, DK)).astype(BF)
        h_c = hidden_state[sl]                           # [BC, D]
        hT = np.ascontiguousarray(h_c.T.reshape(MC, 128, BC).transpose(1, 0, 2))
        in_maps.append({
            "encdr": encdr, "encn": encn, "w1edr": w1edr,
            "w1hT": w1hT, "hT": hT, "b1bc": b1bc, "w2dp": w2dp,
        })
    return in_maps


_NC_CACHE = {}


def kernel(hidden_state, encoder_outputs, W1, b1, W2):
    if "nc" not in _NC_CACHE:
        _NC_CACHE["nc"] = build_kernel(reps=1)
    nc = _NC_CACHE["nc"]
    in_maps = prep_inputs(hidden_state, encoder_outputs, W1, b1, W2)
    res = run_bass_kernel_spmd(nc, in_maps, core_ids=list(range(N_CORES)))
    return np.concatenate([r["out"] for r in res.results], axis=0)
